# revision 10
# baseline (speedup 1.0000x reference)
"""3-layer GCN forward pass on 8 TRN2 NeuronCores.

Strategy (vertex-cut graph parallelism):
  - Each core owns a contiguous block of N/8 destination nodes; edges are
    partitioned by destination so segment sums stay local.
  - Per layer, each GCN conv is computed as (A_hat @ x) @ W + b, i.e.
    aggregate first, then the dense matmul.
  - Aggregation: per-edge rows of the (degree-prescaled) feature matrix are
    fetched with dma_gather; segment-sum is performed on the tensor engine
    as one-hot matmuls accumulating in PSUM (z^T tiles, channels on
    partitions).
  - D^-1/2 normalization is folded per-node: gather reads x_tilde =
    dinv * x (layer 1 folds dinv[src] into the one-hot values instead),
    and the destination-side dinv is applied per-column after aggregation.
    Self loops are plain edges (src == dst), which reproduces the h/deg
    self term exactly.
  - Between layers: the per-core slice of the new features is transposed
    back to node-major (DMA transpose) and AllGathered so every core can
    gather arbitrary source rows next layer.
  - int16 gather indices only reach 32768 rows, so each core's edges are
    split into a "lo" stream (padded row id < 32768) and a "hi" stream,
    gathered from two base offsets; PSUM partials from the two passes are
    combined in SBUF.
"""

import sys
import types

sys.path.insert(0, "/opt/trn_rl_repo")

import numpy as np
import ml_dtypes

import concourse.bass as bass  # noqa: F401
import concourse.bacc as bacc
import concourse.tile as tile
import concourse.mybir as mybir
from concourse import bass_utils
from concourse.masks import make_identity

BF16 = ml_dtypes.bfloat16
FP32 = np.float32


def _install_ntff_hook():
    """The image's antenv lacks axon_hooks; shim it so trace=True works."""
    if "antenv.axon_hooks" in sys.modules:
        return
    mod = types.ModuleType("antenv.axon_hooks")
    mod._hook = None
    mod.set_axon_ntff_profile_hook = lambda h: setattr(mod, "_hook", h)
    mod.get_axon_ntff_profile_hook = lambda: mod._hook
    sys.modules["antenv.axon_hooks"] = mod
    try:
        import antenv

        antenv.axon_hooks = mod
        if "/root/.axon_site" not in sys.path:
            sys.path.insert(0, "/root/.axon_site")
        from trn_agent_boot.trn_boot import _ntff_profile_via_ctypes

        mod.set_axon_ntff_profile_hook(
            _ntff_profile_via_ctypes("/opt/axon/libaxon_pjrt.so"))
    except Exception:
        pass


class Cfg:
    def __init__(self, n=50000, c=128, hid=128, out_c=64, ncores=8,
                 lo_rows=32768, piece_ch=64, sbatch=16):
        self.N = n
        self.C = c
        self.HID = hid
        self.OUT_C = out_c
        self.NCORES = ncores
        self.NPC = n // ncores
        self.TPC = (self.NPC + 127) // 128
        self.NPC_PAD = self.TPC * 128
        self.NPAD = ncores * self.NPC_PAD
        self.LO_ROWS = lo_rows
        self.PIECE_CH = piece_ch
        self.SBATCH = sbatch
        assert n % ncores == 0


FULL = Cfg()
PAD_POS = 240.0


# ---------------------------------------------------------------- host prep
def _preprocess(cfg, edge_index):
    """Compute per-core gather/scatter metadata from the edge list."""
    src = np.asarray(edge_index[0]).astype(np.int64)
    dst = np.asarray(edge_index[1]).astype(np.int64)
    n, npc, npcp, tpc = cfg.N, cfg.NPC, cfg.NPC_PAD, cfg.TPC

    deg = (np.bincount(dst, minlength=n) + 1.0).astype(np.float64)
    dinv = (1.0 / np.sqrt(deg)).astype(np.float32)

    loop = np.arange(n, dtype=np.int64)
    allsrc = np.concatenate([src, loop])
    alldst = np.concatenate([dst, loop])

    m = (allsrc // npc) * npcp + (allsrc % npc)  # padded node-major row
    owner = alldst // npc
    local = alldst - owner * npc
    tile_id = local // 128
    pos = local % 128
    bucket = (m >= cfg.LO_ROWS).astype(np.int64)

    order = np.lexsort((m, bucket, tile_id, owner))
    m_s = m[order]
    owner_s = owner[order]
    tile_s = tile_id[order]
    bucket_s = bucket[order]
    pos_s = pos[order]
    dinvsrc_s = dinv[allsrc[order]]

    # group counts per (core, tile, bucket)
    counts = np.zeros((cfg.NCORES, tpc, 2), dtype=np.int64)
    np.add.at(counts, (owner_s, tile_s, bucket_s), 1)
    clo = np.ceil(counts[:, :, 0] / 128).astype(int).max(axis=0)  # [tpc]
    chi = np.ceil(counts[:, :, 1] / 128).astype(int).max(axis=0)

    # group start offsets in the sorted arrays
    starts = np.zeros((cfg.NCORES, tpc, 2), dtype=np.int64)
    flat_counts = counts.reshape(-1)
    flat_starts = np.concatenate([[0], np.cumsum(flat_counts)[:-1]])
    starts = flat_starts.reshape(cfg.NCORES, tpc, 2)

    nch = int(clo.sum() + chi.sum())
    l_lo = max(int(clo.sum()) * 128, 16)
    l_hi = max(int(chi.sum()) * 128, 16)

    per_core = []
    for c in range(cfg.NCORES):
        idx_lo = np.zeros(l_lo, dtype=np.int32)
        idx_hi = np.zeros(l_hi, dtype=np.int32)
        dpos = np.full((nch, 128), PAD_POS, dtype=np.float32)
        dsrc = np.zeros((nch, 128), dtype=np.float32)

        off_lo = 0
        off_hi = 0
        ch_g = 0  # global chunk counter (lo chunks first)
        for b, cc, idx_arr, base in ((0, clo, idx_lo, 0),
                                     (1, chi, idx_hi, cfg.LO_ROWS)):
            off = 0
            for t in range(tpc):
                s0 = starts[c, t, b]
                cnt = counts[c, t, b]
                cap = cc[t] * 128
                assert cnt <= cap
                idx_arr[off:off + cnt] = m_s[s0:s0 + cnt] - base
                blk = dpos[ch_g:ch_g + cc[t]].reshape(-1)
                blk[:cnt] = pos_s[s0:s0 + cnt]
                blk2 = dsrc[ch_g:ch_g + cc[t]].reshape(-1)
                blk2[:cnt] = dinvsrc_s[s0:s0 + cnt]
                off += cap
                ch_g += cc[t]
        assert ch_g == nch

        def wrap(stream):
            # idx j consumed from [j % 16, j // 16]; replicate to 8 groups
            a = stream.reshape(-1, 16).T.astype(np.int16)
            return np.tile(a, (8, 1)).copy()

        dinv_own = np.zeros(npcp, dtype=np.float32)
        dinv_own[:npc] = dinv[c * npc:(c + 1) * npc]
        per_core.append({
            "idx_lo": wrap(idx_lo),
            "idx_hi": wrap(idx_hi),
            "dstpos": np.ascontiguousarray(dpos.T).astype(BF16),
            "dinvsrc": np.ascontiguousarray(dsrc.T).astype(BF16),
            "dinv_bc": np.tile(dinv_own, (128, 1)).astype(BF16),
        })

    plan = {
        "clo": clo.tolist(),
        "chi": chi.tolist(),
        "nch": nch,
        "l_lo": l_lo,
        "l_hi": l_hi,
    }
    return plan, per_core, dinv


# ------------------------------------------------------------- bass program
def _build(cfg, plan, stage=99):
    clo, chi = plan["clo"], plan["chi"]
    nch, l_lo, l_hi = plan["nch"], plan["l_lo"], plan["l_hi"]
    tpc, npcp, npad = cfg.TPC, cfg.NPC_PAD, cfg.NPAD
    C, OUT_C = cfg.C, cfg.OUT_C
    bf = mybir.dt.bfloat16
    f32 = mybir.dt.float32

    nc = bacc.Bacc("TRN2", target_bir_lowering=False, debug=False,
                   num_devices=cfg.NCORES)

    xt0_d = nc.dram_tensor("xt0", [npad, C], bf, kind="ExternalInput")
    w_d = [nc.dram_tensor(f"w{i}", [C, C if i < 2 else OUT_C], bf,
                          kind="ExternalInput") for i in range(3)]
    b_d = [nc.dram_tensor(f"b{i}", [C if i < 2 else OUT_C, 1], f32,
                          kind="ExternalInput") for i in range(3)]
    idxlo_d = nc.dram_tensor("idx_lo", [128, max(l_lo // 16, 1)],
                             mybir.dt.int16, kind="ExternalInput")
    idxhi_d = nc.dram_tensor("idx_hi", [128, max(l_hi // 16, 1)],
                             mybir.dt.int16, kind="ExternalInput")
    dstpos_d = nc.dram_tensor("dstpos", [128, nch], bf, kind="ExternalInput")
    dinvsrc_d = nc.dram_tensor("dinvsrc", [128, nch], bf,
                               kind="ExternalInput")
    dinvbc_d = nc.dram_tensor("dinv_bc", [128, npcp], bf,
                              kind="ExternalInput")
    iota_d = nc.dram_tensor("iota", [128, 128], bf, kind="ExternalInput")
    out_d = nc.dram_tensor("out", [cfg.NPC, OUT_C], f32,
                           kind="ExternalOutput")

    with tile.TileContext(nc) as tc:
        with (
            tc.tile_pool(name="const", bufs=1) as cpool,
            tc.tile_pool(name="g", bufs=3) as gpool,
            tc.tile_pool(name="s", bufs=4) as spool,
            tc.tile_pool(name="s0", bufs=2) as s0pool,
            tc.tile_pool(name="z", bufs=1) as zpool,
            tc.tile_pool(name="zs", bufs=2) as zspool,
            tc.tile_pool(name="xt", bufs=1) as xtpool,
            tc.tile_pool(name="nm", bufs=1) as nmpool,
            tc.tile_pool(name="fin", bufs=1) as finpool,
            tc.tile_pool(name="psA", bufs=4, space="PSUM") as psa,
            tc.tile_pool(name="psW", bufs=2, space="PSUM") as psw_pool,
            tc.tile_pool(name="psT", bufs=2, space="PSUM") as pst,
            tc.tile_pool(name="dram", bufs=1, space="DRAM") as dpool,
        ):
            # ---- constants into SBUF
            w_sb, b_sb = [], []
            for i in range(3):
                w = cpool.tile([C, C if i < 2 else OUT_C], bf, name=f"wt{i}")
                nc.sync.dma_start(w[:], w_d[i][:])
                bt = cpool.tile([C if i < 2 else OUT_C, 1], f32, name=f"bt{i}")
                nc.sync.dma_start(bt[:], b_d[i][:])
                w_sb.append(w)
                b_sb.append(bt)
            idxlo_sb = cpool.tile([128, max(l_lo // 16, 1)], mybir.dt.int16,
                                  tag="idxlo")
            nc.sync.dma_start(idxlo_sb[:], idxlo_d[:])
            idxhi_sb = cpool.tile([128, max(l_hi // 16, 1)], mybir.dt.int16,
                                  tag="idxhi")
            nc.sync.dma_start(idxhi_sb[:], idxhi_d[:])
            dstpos_sb = cpool.tile([128, nch], bf, tag="dstpos")
            nc.sync.dma_start(dstpos_sb[:], dstpos_d[:])
            dinvsrc_sb = cpool.tile([128, nch], bf, tag="dinvsrc")
            nc.sync.dma_start(dinvsrc_sb[:], dinvsrc_d[:])
            dinvbc_sb = cpool.tile([128, npcp], bf, tag="dinvbc")
            nc.sync.dma_start(dinvbc_sb[:], dinvbc_d[:])
            iota_sb = cpool.tile([128, 128], bf, tag="iota")
            nc.sync.dma_start(iota_sb[:], iota_d[:])
            ident = cpool.tile([OUT_C, OUT_C], f32, tag="ident")
            make_identity(nc, ident[:])

            # AllGather buffers (bf16 node-major, padded layout)
            ag_in = [dpool.tile([npcp, C], bf, name=f"ag_in{i}")
                     for i in range(2)]
            xt_full = [dpool.tile([npad, C], bf,
                                  name=f"xt_full{i}") for i in range(2)]

            lo_total, hi_total = sum(clo), sum(chi)

            if stage < 99:
                nfull0 = cfg.NPC // 128
                dummy = cpool.tile([128, nfull0, 64], f32, tag="dummy")
                nc.gpsimd.memset(dummy[:], 0.0)
                nc.sync.dma_start(
                    out_d[:nfull0 * 128].rearrange("(t p) c -> p t c", p=128),
                    dummy[:])
                if cfg.NPC - nfull0 * 128:
                    nc.sync.dma_start(out_d[nfull0 * 128:cfg.NPC],
                                      dummy[:cfg.NPC - nfull0 * 128, 0, :])

            nlayers = (0 if stage <= 0 else
                       1 if stage <= 5 else (2 if stage == 6 else 3))
            for layer in range(nlayers):
                src_dram = xt0_d[:] if layer == 0 else xt_full[layer - 1][:]
                cout = C if layer < 2 else OUT_C

                # ---- gathers (lo stream then hi stream)
                g_slots = []
                for total, idx_sb, base in ((lo_total, idxlo_sb, 0),
                                            (hi_total, idxhi_sb,
                                             cfg.LO_ROWS)):
                    c0 = 0
                    while c0 < total:
                        pch = min(cfg.PIECE_CH, total - c0)
                        g = gpool.tile([128, pch, C], bf, tag="g")
                        nrows = (npad - base if base + 32768 > npad
                                 else 32768)
                        nc.gpsimd.dma_gather(
                            g[:],
                            src_dram[base:base + nrows, :],
                            idx_sb[:, c0 * 8:(c0 + pch) * 8],
                            pch * 128,
                            pch * 128,
                            C,
                            single_packet=False,
                        )
                        for k in range(pch):
                            g_slots.append((g, k))
                        c0 += pch

                if stage <= 1:
                    continue
                # ---- S tiles (one-hot / dinv-hot) in combined chunk order
                s_slots = []
                c0 = 0
                while c0 < nch:
                    sb = min(cfg.SBATCH, nch - c0)
                    shp = [128, sb, 128]
                    s = spool.tile(shp, bf, tag="s")
                    iota_v = iota_sb[:].unsqueeze(1).broadcast_to(shp)
                    dp_v = dstpos_sb[:, c0:c0 + sb].unsqueeze(2) \
                        .broadcast_to(shp)
                    if layer == 0:
                        s0 = s0pool.tile(shp, bf, tag="s0")
                        nc.vector.tensor_tensor(s0[:], iota_v, dp_v,
                                                mybir.AluOpType.is_equal)
                        dv_v = dinvsrc_sb[:, c0:c0 + sb].unsqueeze(2) \
                            .broadcast_to(shp)
                        nc.vector.tensor_tensor(s[:], s0[:], dv_v,
                                                mybir.AluOpType.mult)
                    else:
                        nc.vector.tensor_tensor(s[:], iota_v, dp_v,
                                                mybir.AluOpType.is_equal)
                    for k in range(sb):
                        s_slots.append((s, k))
                    c0 += sb

                if stage <= 2:
                    continue
                # ---- segment-sum matmuls, accumulating z^T per dst tile
                z = zpool.tile([128, npcp], f32, tag="z")
                gi = 0
                for phase, cc in ((0, clo), (1, chi)):
                    for t in range(tpc):
                        cnt = cc[t]
                        sl = np.s_[:, t * 128:(t + 1) * 128]
                        if cnt == 0:
                            if phase == 0 and chi[t] == 0:
                                nc.vector.memset(z[sl], 0.0)
                            continue
                        ps = psa.tile([128, 128], f32, tag="psA")
                        for k in range(cnt):
                            g, gk = g_slots[gi]
                            s, sk = s_slots[gi]
                            gi += 1
                            nc.tensor.matmul(ps[:], g[:, gk, :], s[:, sk, :],
                                             start=(k == 0),
                                             stop=(k == cnt - 1))
                        if phase == 0:
                            nc.scalar.copy(z[sl], ps[:])
                        elif clo[t] == 0:
                            nc.scalar.copy(z[sl], ps[:])
                        else:
                            nc.vector.tensor_add(z[sl], z[sl], ps[:])
                assert gi == nch

                if stage <= 3:
                    continue
                # ---- dinv[dst] column scale, W matmul, bias/relu
                if layer < 2:
                    xt = xtpool.tile([128, npcp], bf, tag="xt")
                else:
                    fin = finpool.tile([OUT_C, npcp], f32, tag="fin")

                nblk = [(i * 512, min(512, npcp - i * 512))
                        for i in range((npcp + 511) // 512)]
                for bo, bs in nblk:
                    sl = np.s_[:, bo:bo + bs]
                    zs = zspool.tile([128, 512], bf, tag="zs")
                    nc.vector.tensor_tensor(zs[:, :bs], z[sl], dinvbc_sb[sl],
                                            mybir.AluOpType.mult)
                    psw = psw_pool.tile([cout, 512], f32, tag="psW")
                    nc.tensor.matmul(psw[:, :bs], w_sb[layer][:],
                                     zs[:, :bs], start=True, stop=True)
                    if layer < 2:
                        tmp = zspool.tile([128, 512], bf, tag="acttmp")
                        nc.scalar.activation(
                            tmp[:, :bs], psw[:, :bs],
                            mybir.ActivationFunctionType.Relu,
                            bias=b_sb[layer][:])
                        nc.vector.tensor_tensor(xt[sl], tmp[:, :bs],
                                                dinvbc_sb[sl],
                                                mybir.AluOpType.mult)
                    else:
                        nc.scalar.activation(
                            fin[sl], psw[:cout, :bs],
                            mybir.ActivationFunctionType.Identity,
                            bias=b_sb[layer][:])

                if stage <= 4:
                    continue
                if layer < 2:
                    # node-major transpose + AllGather
                    xt_nm = nmpool.tile([128, tpc, C], bf, tag="nm")
                    nc.sync.dma_start_transpose(xt_nm[:], xt[:])
                    nc.sync.dma_start(
                        ag_in[layer][:].rearrange("(t p) c -> p t c", p=128),
                        xt_nm[:])
                    nc.gpsimd.collective_compute(
                        "AllGather",
                        mybir.AluOpType.bypass,
                        replica_groups=[list(range(cfg.NCORES))],
                        ins=[ag_in[layer].opt()],
                        outs=[xt_full[layer].opt()],
                    )
                else:
                    # final: transpose 64xN^T -> node-major fp32, DMA out
                    out_nm = finpool.tile([128, tpc, OUT_C], f32, tag="onm")
                    for t in range(tpc):
                        tp = pst.tile([128, OUT_C], f32, tag="psT")
                        nc.tensor.transpose(
                            tp[:], fin[:, t * 128:(t + 1) * 128], ident[:])
                        nc.scalar.copy(out_nm[:, t, :], tp[:])
                    nfull = cfg.NPC // 128
                    rem = cfg.NPC - nfull * 128
                    nc.sync.dma_start(
                        out_d[:nfull * 128].rearrange("(t p) c -> p t c",
                                                      p=128),
                        out_nm[:, :nfull, :])
                    if rem:
                        nc.sync.dma_start(out_d[nfull * 128:cfg.NPC],
                                          out_nm[:rem, nfull, :])

    nc.compile()
    return nc


# ------------------------------------------------------------------ driver
_CACHE = {}


def _get_program(cfg, plan, stage=99):
    key = (cfg.N, cfg.NCORES, stage, tuple(plan["clo"]), tuple(plan["chi"]))
    if key not in _CACHE:
        _CACHE[key] = _build(cfg, plan, stage)
    return _CACHE[key]


def _make_in_maps(cfg, x, weights, biases, plan, per_core):
    x = np.asarray(x, dtype=np.float32)
    npc, npcp = cfg.NPC, cfg.NPC_PAD
    xt0 = np.zeros((cfg.NPAD, cfg.C), dtype=BF16)
    for r in range(cfg.NCORES):
        xt0[r * npcp:r * npcp + npc] = x[r * npc:(r + 1) * npc].astype(BF16)
    iota = np.tile(np.arange(128, dtype=np.float32), (128, 1)).astype(BF16)

    in_maps = []
    for c in range(cfg.NCORES):
        m = {
            "xt0": xt0,
            "iota": iota,
            "idx_lo": per_core[c]["idx_lo"],
            "idx_hi": per_core[c]["idx_hi"],
            "dstpos": per_core[c]["dstpos"],
            "dinvsrc": per_core[c]["dinvsrc"],
            "dinv_bc": per_core[c]["dinv_bc"],
        }
        for i in range(3):
            m[f"w{i}"] = np.asarray(weights[i], dtype=np.float32) \
                .astype(BF16)
            m[f"b{i}"] = np.asarray(biases[i], dtype=np.float32) \
                .reshape(-1, 1)
        in_maps.append(m)
    return in_maps


def run(cfg, x, edge_index, weights, biases, sim=False, trace=False,
        stage=99):
    plan, per_core, _ = _preprocess(cfg, edge_index)
    nc = _get_program(cfg, plan, stage)
    in_maps = _make_in_maps(cfg, x, weights, biases, plan, per_core)

    if sim:
        from concourse.bass_interp import MultiCoreSim

        s = MultiCoreSim(nc, num_cores=cfg.NCORES, num_workers=1)
        for c in range(cfg.NCORES):
            for k, v in in_maps[c].items():
                s.cores[c].tensor(k)[:] = v
        s.simulate()
        results = [{"out": s.cores[c].tensor("out").copy()}
                   for c in range(cfg.NCORES)]
        res = None
    else:
        _install_ntff_hook()
        res = bass_utils.run_bass_kernel_spmd(
            nc, in_maps, core_ids=list(range(cfg.NCORES)), trace=trace)
        results = res.results

    out = np.concatenate([results[c]["out"] for c in range(cfg.NCORES)], 0)
    return out, res


def kernel(x, edge_index, W1, b1, W2, b2, W3, b3):
    out, _ = run(FULL, x, edge_index, (W1, W2, W3), (b1, b2, b3))
    return out


# revision 11
# speedup vs baseline: 1.6736x; 1.6736x over previous
"""3-layer GCN forward pass on 8 TRN2 NeuronCores.

Strategy (vertex-cut graph parallelism):
  - Each core owns a contiguous block of N/8 destination nodes; edges are
    partitioned by destination so segment sums stay local.
  - Per layer, each GCN conv is computed as (A_hat @ x) @ W + b, i.e.
    aggregate first, then the dense matmul.
  - Aggregation: per-edge rows of the (degree-prescaled) feature matrix are
    fetched with dma_gather; segment-sum is performed on the tensor engine
    as one-hot matmuls accumulating in PSUM (z^T tiles, channels on
    partitions).
  - D^-1/2 normalization is folded per-node: gather reads x_tilde =
    dinv * x (layer 1 folds dinv[src] into the one-hot values instead),
    and the destination-side dinv is applied per-column after aggregation.
    Self loops are plain edges (src == dst), which reproduces the h/deg
    self term exactly.
  - Between layers: the per-core slice of the new features is transposed
    back to node-major (DMA transpose) and AllGathered so every core can
    gather arbitrary source rows next layer.
  - int16 gather indices only reach 32768 rows, so each core's edges are
    split into a "lo" stream (padded row id < 32768) and a "hi" stream,
    gathered from two base offsets; PSUM partials from the two passes are
    combined in SBUF.
"""

import sys
import types

sys.path.insert(0, "/opt/trn_rl_repo")

import numpy as np
import ml_dtypes

import concourse.bass as bass  # noqa: F401
import concourse.bacc as bacc
import concourse.tile as tile
import concourse.mybir as mybir
from concourse import bass_utils
from concourse.masks import make_identity

BF16 = ml_dtypes.bfloat16
FP32 = np.float32


def _install_ntff_hook():
    """The image's antenv lacks axon_hooks; shim it so trace=True works."""
    if "antenv.axon_hooks" in sys.modules:
        return
    mod = types.ModuleType("antenv.axon_hooks")
    mod._hook = None
    mod.set_axon_ntff_profile_hook = lambda h: setattr(mod, "_hook", h)
    mod.get_axon_ntff_profile_hook = lambda: mod._hook
    sys.modules["antenv.axon_hooks"] = mod
    try:
        import antenv

        antenv.axon_hooks = mod
        if "/root/.axon_site" not in sys.path:
            sys.path.insert(0, "/root/.axon_site")
        from trn_agent_boot.trn_boot import _ntff_profile_via_ctypes

        mod.set_axon_ntff_profile_hook(
            _ntff_profile_via_ctypes("/opt/axon/libaxon_pjrt.so"))
    except Exception:
        pass


class Cfg:
    def __init__(self, n=50000, c=128, hid=128, out_c=64, ncores=8,
                 lo_rows=32768, piece_ch=64, sbatch=16):
        self.N = n
        self.C = c
        self.HID = hid
        self.OUT_C = out_c
        self.NCORES = ncores
        self.NPC = n // ncores
        self.TPC = (self.NPC + 127) // 128
        self.NPC_PAD = self.TPC * 128
        self.NPAD = ncores * self.NPC_PAD
        self.LO_ROWS = lo_rows
        self.PIECE_CH = piece_ch
        self.SBATCH = sbatch
        assert n % ncores == 0


FULL = Cfg()
PAD_POS = 240.0


# ---------------------------------------------------------------- host prep
def _preprocess(cfg, edge_index):
    """Compute per-core gather/scatter metadata from the edge list."""
    src = np.asarray(edge_index[0]).astype(np.int64)
    dst = np.asarray(edge_index[1]).astype(np.int64)
    n, npc, npcp, tpc = cfg.N, cfg.NPC, cfg.NPC_PAD, cfg.TPC

    deg = (np.bincount(dst, minlength=n) + 1.0).astype(np.float64)
    dinv = (1.0 / np.sqrt(deg)).astype(np.float32)

    loop = np.arange(n, dtype=np.int64)
    allsrc = np.concatenate([src, loop])
    alldst = np.concatenate([dst, loop])

    m = (allsrc // npc) * npcp + (allsrc % npc)  # padded node-major row
    owner = alldst // npc
    local = alldst - owner * npc
    tile_id = local // 128
    pos = local % 128
    bucket = (m >= cfg.LO_ROWS).astype(np.int64)

    order = np.lexsort((m, bucket, tile_id, owner))
    m_s = m[order]
    owner_s = owner[order]
    tile_s = tile_id[order]
    bucket_s = bucket[order]
    pos_s = pos[order]
    dinvsrc_s = dinv[allsrc[order]]

    # group counts per (core, tile, bucket)
    counts = np.zeros((cfg.NCORES, tpc, 2), dtype=np.int64)
    np.add.at(counts, (owner_s, tile_s, bucket_s), 1)
    clo = np.ceil(counts[:, :, 0] / 128).astype(int).max(axis=0)  # [tpc]
    chi = np.ceil(counts[:, :, 1] / 128).astype(int).max(axis=0)

    # group start offsets in the sorted arrays
    starts = np.zeros((cfg.NCORES, tpc, 2), dtype=np.int64)
    flat_counts = counts.reshape(-1)
    flat_starts = np.concatenate([[0], np.cumsum(flat_counts)[:-1]])
    starts = flat_starts.reshape(cfg.NCORES, tpc, 2)

    nch = int(clo.sum() + chi.sum())
    l_lo = max(int(clo.sum()) * 128, 16)
    l_hi = max(int(chi.sum()) * 128, 16)

    per_core = []
    for c in range(cfg.NCORES):
        idx_lo = np.zeros(l_lo, dtype=np.int32)
        idx_hi = np.zeros(l_hi, dtype=np.int32)
        dpos = np.full((nch, 128), PAD_POS, dtype=np.float32)
        dsrc = np.zeros((nch, 128), dtype=np.float32)

        off_lo = 0
        off_hi = 0
        ch_g = 0  # global chunk counter (lo chunks first)
        for b, cc, idx_arr, base in ((0, clo, idx_lo, 0),
                                     (1, chi, idx_hi, cfg.LO_ROWS)):
            off = 0
            for t in range(tpc):
                s0 = starts[c, t, b]
                cnt = counts[c, t, b]
                cap = cc[t] * 128
                assert cnt <= cap
                idx_arr[off:off + cnt] = m_s[s0:s0 + cnt] - base
                blk = dpos[ch_g:ch_g + cc[t]].reshape(-1)
                blk[:cnt] = pos_s[s0:s0 + cnt]
                blk2 = dsrc[ch_g:ch_g + cc[t]].reshape(-1)
                blk2[:cnt] = dinvsrc_s[s0:s0 + cnt]
                off += cap
                ch_g += cc[t]
        assert ch_g == nch

        def wrap(stream):
            # idx j consumed from [j % 16, j // 16]; replicate to 8 groups
            a = stream.reshape(-1, 16).T.astype(np.int16)
            return np.tile(a, (8, 1)).copy()

        dinv_own = np.zeros(npcp, dtype=np.float32)
        dinv_own[:npc] = dinv[c * npc:(c + 1) * npc]
        per_core.append({
            "idx_lo": wrap(idx_lo),
            "idx_hi": wrap(idx_hi),
            "dstpos": np.ascontiguousarray(dpos.T).astype(BF16),
            "dinvsrc": np.ascontiguousarray(dsrc.T).astype(BF16),
            "dinv_bc": np.tile(dinv_own, (128, 1)).astype(BF16),
        })

    plan = {
        "clo": clo.tolist(),
        "chi": chi.tolist(),
        "nch": nch,
        "l_lo": l_lo,
        "l_hi": l_hi,
    }
    return plan, per_core, dinv


# ------------------------------------------------------------- bass program
def _build(cfg, plan, stage=99):
    clo, chi = plan["clo"], plan["chi"]
    nch, l_lo, l_hi = plan["nch"], plan["l_lo"], plan["l_hi"]
    tpc, npcp, npad = cfg.TPC, cfg.NPC_PAD, cfg.NPAD
    C, OUT_C = cfg.C, cfg.OUT_C
    bf = mybir.dt.bfloat16
    f32 = mybir.dt.float32

    nc = bacc.Bacc("TRN2", target_bir_lowering=False, debug=False,
                   num_devices=cfg.NCORES, num_swdge_queues=4)

    xt0_d = nc.dram_tensor("xt0", [npad, C], bf, kind="ExternalInput")
    w_d = [nc.dram_tensor(f"w{i}", [C, C if i < 2 else OUT_C], bf,
                          kind="ExternalInput") for i in range(3)]
    b_d = [nc.dram_tensor(f"b{i}", [C if i < 2 else OUT_C, 1], f32,
                          kind="ExternalInput") for i in range(3)]
    idxlo_d = nc.dram_tensor("idx_lo", [128, max(l_lo // 16, 1)],
                             mybir.dt.int16, kind="ExternalInput")
    idxhi_d = nc.dram_tensor("idx_hi", [128, max(l_hi // 16, 1)],
                             mybir.dt.int16, kind="ExternalInput")
    dstpos_d = nc.dram_tensor("dstpos", [128, nch], bf, kind="ExternalInput")
    dinvsrc_d = nc.dram_tensor("dinvsrc", [128, nch], bf,
                               kind="ExternalInput")
    dinvbc_d = nc.dram_tensor("dinv_bc", [128, npcp], bf,
                              kind="ExternalInput")
    iota_d = nc.dram_tensor("iota", [128, 128], bf, kind="ExternalInput")
    out_d = nc.dram_tensor("out", [cfg.NPC, OUT_C], f32,
                           kind="ExternalOutput")

    with tile.TileContext(nc) as tc:
        with (
            tc.tile_pool(name="const", bufs=1) as cpool,
            tc.tile_pool(name="g", bufs=3) as gpool,
            tc.tile_pool(name="s", bufs=4) as spool,
            tc.tile_pool(name="s0", bufs=2) as s0pool,
            tc.tile_pool(name="z", bufs=1) as zpool,
            tc.tile_pool(name="zs", bufs=2) as zspool,
            tc.tile_pool(name="xt", bufs=1) as xtpool,
            tc.tile_pool(name="nm", bufs=1) as nmpool,
            tc.tile_pool(name="fin", bufs=1) as finpool,
            tc.tile_pool(name="psA", bufs=4, space="PSUM") as psa,
            tc.tile_pool(name="psW", bufs=2, space="PSUM") as psw_pool,
            tc.tile_pool(name="psT", bufs=2, space="PSUM") as pst,
            tc.tile_pool(name="dram", bufs=1, space="DRAM") as dpool,
        ):
            # ---- constants into SBUF
            w_sb, b_sb = [], []
            for i in range(3):
                w = cpool.tile([C, C if i < 2 else OUT_C], bf, name=f"wt{i}")
                nc.sync.dma_start(w[:], w_d[i][:])
                bt = cpool.tile([C if i < 2 else OUT_C, 1], f32, name=f"bt{i}")
                nc.sync.dma_start(bt[:], b_d[i][:])
                w_sb.append(w)
                b_sb.append(bt)
            idxlo_sb = cpool.tile([128, max(l_lo // 16, 1)], mybir.dt.int16,
                                  tag="idxlo")
            nc.sync.dma_start(idxlo_sb[:], idxlo_d[:])
            idxhi_sb = cpool.tile([128, max(l_hi // 16, 1)], mybir.dt.int16,
                                  tag="idxhi")
            nc.sync.dma_start(idxhi_sb[:], idxhi_d[:])
            dstpos_sb = cpool.tile([128, nch], bf, tag="dstpos")
            nc.sync.dma_start(dstpos_sb[:], dstpos_d[:])
            dinvsrc_sb = cpool.tile([128, nch], bf, tag="dinvsrc")
            nc.sync.dma_start(dinvsrc_sb[:], dinvsrc_d[:])
            dinvbc_sb = cpool.tile([128, npcp], bf, tag="dinvbc")
            nc.sync.dma_start(dinvbc_sb[:], dinvbc_d[:])
            iota_sb = cpool.tile([128, 128], bf, tag="iota")
            nc.sync.dma_start(iota_sb[:], iota_d[:])
            ident = cpool.tile([OUT_C, OUT_C], f32, tag="ident")
            make_identity(nc, ident[:])

            # AllGather buffers (bf16 node-major, padded layout)
            ag_in = [dpool.tile([npcp, C], bf, name=f"ag_in{i}")
                     for i in range(2)]
            xt_full = [dpool.tile([npad, C], bf,
                                  name=f"xt_full{i}") for i in range(2)]

            lo_total, hi_total = sum(clo), sum(chi)

            if stage < 99:
                nfull0 = cfg.NPC // 128
                dummy = cpool.tile([128, nfull0, 64], f32, tag="dummy")
                nc.gpsimd.memset(dummy[:], 0.0)
                nc.sync.dma_start(
                    out_d[:nfull0 * 128].rearrange("(t p) c -> p t c", p=128),
                    dummy[:])
                if cfg.NPC - nfull0 * 128:
                    nc.sync.dma_start(out_d[nfull0 * 128:cfg.NPC],
                                      dummy[:cfg.NPC - nfull0 * 128, 0, :])

            nlayers = (0 if stage <= 0 else
                       1 if stage <= 5 else (2 if stage == 6 else 3))
            for layer in range(nlayers):
                src_dram = xt0_d[:] if layer == 0 else xt_full[layer - 1][:]
                cout = C if layer < 2 else OUT_C

                # ---- gathers (lo stream then hi stream), round-robin
                # over the 4 SWDGE queues = 4 Q7 core pairs in parallel
                g_slots = []
                qrr = 0
                for total, idx_sb, base in ((lo_total, idxlo_sb, 0),
                                            (hi_total, idxhi_sb,
                                             cfg.LO_ROWS)):
                    c0 = 0
                    while c0 < total:
                        pch = min(cfg.PIECE_CH, total - c0)
                        g = gpool.tile([128, pch, C], bf, tag="g")
                        nrows = (npad - base if base + 32768 > npad
                                 else 32768)
                        nc.gpsimd.dma_gather(
                            g[:],
                            src_dram[base:base + nrows, :],
                            idx_sb[:, c0 * 8:(c0 + pch) * 8],
                            pch * 128,
                            pch * 128,
                            C,
                            single_packet=False,
                            queue_num=qrr % 4,
                        )
                        qrr += 1
                        for k in range(pch):
                            g_slots.append((g, k))
                        c0 += pch

                if stage <= 1:
                    continue
                # ---- S tiles (one-hot / dinv-hot) in combined chunk order
                s_slots = []
                c0 = 0
                while c0 < nch:
                    sb = min(cfg.SBATCH, nch - c0)
                    shp = [128, sb, 128]
                    s = spool.tile(shp, bf, tag="s")
                    iota_v = iota_sb[:].unsqueeze(1).broadcast_to(shp)
                    dp_v = dstpos_sb[:, c0:c0 + sb].unsqueeze(2) \
                        .broadcast_to(shp)
                    if layer == 0:
                        s0 = s0pool.tile(shp, bf, tag="s0")
                        nc.vector.tensor_tensor(s0[:], iota_v, dp_v,
                                                mybir.AluOpType.is_equal)
                        dv_v = dinvsrc_sb[:, c0:c0 + sb].unsqueeze(2) \
                            .broadcast_to(shp)
                        nc.vector.tensor_tensor(s[:], s0[:], dv_v,
                                                mybir.AluOpType.mult)
                    else:
                        nc.vector.tensor_tensor(s[:], iota_v, dp_v,
                                                mybir.AluOpType.is_equal)
                    for k in range(sb):
                        s_slots.append((s, k))
                    c0 += sb

                if stage <= 2:
                    continue
                # ---- segment-sum matmuls, accumulating z^T per dst tile
                z = zpool.tile([128, npcp], f32, tag="z")
                gi = 0
                for phase, cc in ((0, clo), (1, chi)):
                    for t in range(tpc):
                        cnt = cc[t]
                        sl = np.s_[:, t * 128:(t + 1) * 128]
                        if cnt == 0:
                            if phase == 0 and chi[t] == 0:
                                nc.vector.memset(z[sl], 0.0)
                            continue
                        ps = psa.tile([128, 128], f32, tag="psA")
                        for k in range(cnt):
                            g, gk = g_slots[gi]
                            s, sk = s_slots[gi]
                            gi += 1
                            nc.tensor.matmul(ps[:], g[:, gk, :], s[:, sk, :],
                                             start=(k == 0),
                                             stop=(k == cnt - 1))
                        if phase == 0:
                            nc.scalar.copy(z[sl], ps[:])
                        elif clo[t] == 0:
                            nc.scalar.copy(z[sl], ps[:])
                        else:
                            nc.vector.tensor_add(z[sl], z[sl], ps[:])
                assert gi == nch

                if stage <= 3:
                    continue
                # ---- dinv[dst] column scale, W matmul, bias/relu
                if layer < 2:
                    xt = xtpool.tile([128, npcp], bf, tag="xt")
                else:
                    fin = finpool.tile([OUT_C, npcp], f32, tag="fin")

                nblk = [(i * 512, min(512, npcp - i * 512))
                        for i in range((npcp + 511) // 512)]
                for bo, bs in nblk:
                    sl = np.s_[:, bo:bo + bs]
                    zs = zspool.tile([128, 512], bf, tag="zs")
                    nc.vector.tensor_tensor(zs[:, :bs], z[sl], dinvbc_sb[sl],
                                            mybir.AluOpType.mult)
                    psw = psw_pool.tile([cout, 512], f32, tag="psW")
                    nc.tensor.matmul(psw[:, :bs], w_sb[layer][:],
                                     zs[:, :bs], start=True, stop=True)
                    if layer < 2:
                        tmp = zspool.tile([128, 512], bf, tag="acttmp")
                        nc.scalar.activation(
                            tmp[:, :bs], psw[:, :bs],
                            mybir.ActivationFunctionType.Relu,
                            bias=b_sb[layer][:])
                        nc.vector.tensor_tensor(xt[sl], tmp[:, :bs],
                                                dinvbc_sb[sl],
                                                mybir.AluOpType.mult)
                    else:
                        nc.scalar.activation(
                            fin[sl], psw[:cout, :bs],
                            mybir.ActivationFunctionType.Identity,
                            bias=b_sb[layer][:])

                if stage <= 4:
                    continue
                if layer < 2:
                    # node-major transpose + AllGather
                    xt_nm = nmpool.tile([128, tpc, C], bf, tag="nm")
                    nc.sync.dma_start_transpose(xt_nm[:], xt[:])
                    nc.sync.dma_start(
                        ag_in[layer][:].rearrange("(t p) c -> p t c", p=128),
                        xt_nm[:])
                    nc.gpsimd.collective_compute(
                        "AllGather",
                        mybir.AluOpType.bypass,
                        replica_groups=[list(range(cfg.NCORES))],
                        ins=[ag_in[layer].opt()],
                        outs=[xt_full[layer].opt()],
                    )
                else:
                    # final: transpose 64xN^T -> node-major fp32, DMA out
                    out_nm = finpool.tile([128, tpc, OUT_C], f32, tag="onm")
                    for t in range(tpc):
                        tp = pst.tile([128, OUT_C], f32, tag="psT")
                        nc.tensor.transpose(
                            tp[:], fin[:, t * 128:(t + 1) * 128], ident[:])
                        nc.scalar.copy(out_nm[:, t, :], tp[:])
                    nfull = cfg.NPC // 128
                    rem = cfg.NPC - nfull * 128
                    nc.sync.dma_start(
                        out_d[:nfull * 128].rearrange("(t p) c -> p t c",
                                                      p=128),
                        out_nm[:, :nfull, :])
                    if rem:
                        nc.sync.dma_start(out_d[nfull * 128:cfg.NPC],
                                          out_nm[:rem, nfull, :])

    nc.compile()
    return nc


# ------------------------------------------------------------------ driver
_CACHE = {}


def _get_program(cfg, plan, stage=99):
    key = (cfg.N, cfg.NCORES, stage, tuple(plan["clo"]), tuple(plan["chi"]))
    if key not in _CACHE:
        _CACHE[key] = _build(cfg, plan, stage)
    return _CACHE[key]


def _make_in_maps(cfg, x, weights, biases, plan, per_core):
    x = np.asarray(x, dtype=np.float32)
    npc, npcp = cfg.NPC, cfg.NPC_PAD
    xt0 = np.zeros((cfg.NPAD, cfg.C), dtype=BF16)
    for r in range(cfg.NCORES):
        xt0[r * npcp:r * npcp + npc] = x[r * npc:(r + 1) * npc].astype(BF16)
    iota = np.tile(np.arange(128, dtype=np.float32), (128, 1)).astype(BF16)

    in_maps = []
    for c in range(cfg.NCORES):
        m = {
            "xt0": xt0,
            "iota": iota,
            "idx_lo": per_core[c]["idx_lo"],
            "idx_hi": per_core[c]["idx_hi"],
            "dstpos": per_core[c]["dstpos"],
            "dinvsrc": per_core[c]["dinvsrc"],
            "dinv_bc": per_core[c]["dinv_bc"],
        }
        for i in range(3):
            m[f"w{i}"] = np.asarray(weights[i], dtype=np.float32) \
                .astype(BF16)
            m[f"b{i}"] = np.asarray(biases[i], dtype=np.float32) \
                .reshape(-1, 1)
        in_maps.append(m)
    return in_maps


def run(cfg, x, edge_index, weights, biases, sim=False, trace=False,
        stage=99):
    plan, per_core, _ = _preprocess(cfg, edge_index)
    nc = _get_program(cfg, plan, stage)
    in_maps = _make_in_maps(cfg, x, weights, biases, plan, per_core)

    if sim:
        from concourse.bass_interp import MultiCoreSim

        s = MultiCoreSim(nc, num_cores=cfg.NCORES, num_workers=1)
        for c in range(cfg.NCORES):
            for k, v in in_maps[c].items():
                s.cores[c].tensor(k)[:] = v
        s.simulate()
        results = [{"out": s.cores[c].tensor("out").copy()}
                   for c in range(cfg.NCORES)]
        res = None
    else:
        _install_ntff_hook()
        res = bass_utils.run_bass_kernel_spmd(
            nc, in_maps, core_ids=list(range(cfg.NCORES)), trace=trace)
        results = res.results

    out = np.concatenate([results[c]["out"] for c in range(cfg.NCORES)], 0)
    return out, res


def kernel(x, edge_index, W1, b1, W2, b2, W3, b3):
    out, _ = run(FULL, x, edge_index, (W1, W2, W3), (b1, b2, b3))
    return out
